# revision 1
# baseline (speedup 1.0000x reference)
"""Trainium2 Bass kernel for nn_BCCLayer (bilinear co-attention + pooling + batchnorm).

Math
----
The reference computes, per batch b, two bilinear attention maps
G = (relu(P@Wq^T+Qb)*h_mat) @ relu(R@Wk^T+Kb)^T  of shape [2000, 2000],
applies a masked softmax over the first (u) axis, contracts with the
V-side features, mean-pools over the sequence, and batchnorms over the
batch. Because the softmax mask depends only on the column index and the
softmax normalizes over rows, the per-element attention weights are never
needed — only two column sums of exp(G):

  S_all[q] = sum_u exp(G[u,q])
  S_w[q]   = sum_u mask_p[u] * exp(G[u,q])
  w[q]     = mask_v[q]/L * S_w[q]/S_all[q]
  contrib[k] = sum_q w[q] * V[q,k]

(any per-column shift of G — including h_bias — cancels in the ratio,
and |G| < ~1 so exp needs no max-subtraction).

Numerics: w is extremely robust (errors average over 2000-term sums and
mostly cancel in the S_w/S_all ratio), so the attention-map pipeline
(FC-T + G) runs in fp8e4 DoubleRow at 2x MACs. The value chain
(Vnat = relu(R@Wk^T+Kb), contrib = w@Vnat) feeds the batchnorm whose
across-batch variance is tiny (~6e-4 vs 0.25 scale, a ~400x error
amplifier), so it stays in fp32r (full-rate fp32 matmul mode).

Sharding: 8 independent (batch, map) units -> one per NeuronCore, SPMD;
the [4,512] batchnorm epilogue runs on host (the only cross-core step).

fp8 scaling: W x64 (its ~3e-3 entries would be subnormal in e4m3), so
FC psums and the relu'd features carry a 64x scale; exp() applies the
1/64^2 correction via the ACT affine. P and R ship as host-prepared fp8
h-pairs packed in uint16 so the XBAR DMA-transpose (2-byte-only) yields
ready fp8 operands whose pair dim is the DoubleRow interleave.

Only q columns with mask_v > 0 contribute to the output, so the host
permutes valid columns to the front (per core) and the computed q window
shrinks to ceil(max_valid/512) 512-col chunks (2..4), chosen at runtime
from the actual masks — ~35% faster for 50%-dense masks.
"""

import numpy as np

L = 2000
LP = 2048  # L padded to a multiple of 512
HD = 256
KD = 512
B = 4
EPS = 1e-5
NCORES = 8
WSCALE = 64.0   # fp8 weight scale

_NC_CACHE = {}


def _build_nc(nqch=4):
    import concourse.mybir as mybir
    import concourse.tile as tile
    from concourse import bacc

    f32 = mybir.dt.float32
    bf16 = mybir.dt.bfloat16
    fp8 = mybir.dt.float8e4
    f32r = mybir.dt.float32r
    AF = mybir.ActivationFunctionType
    DR = mybir.MatmulPerfMode.DoubleRow

    nc = bacc.Bacc("TRN2", target_bir_lowering=False)

    u16 = mybir.dt.uint16
    NQCh = nqch             # packed q window in 512-col chunks (valid cols first)
    NQP = 512 * NQCh
    NQT = NQP // 128
    # fp8(P), fp8(R) packed as h-pairs in uint16 so the XBAR can transpose them
    p8_in = nc.dram_tensor("p8_in", [LP, HD // 2], u16, kind="ExternalInput")
    r8_in = nc.dram_tensor("r8_in", [NQP, HD // 2], u16, kind="ExternalInput")
    r_f32 = nc.dram_tensor("r_f32", [NQP, HD], f32, kind="ExternalInput")
    # 64*W^T in the matching (c, s, k) interleaved order, already fp8
    wq8_in = nc.dram_tensor("wq8_in", [128, 2, KD], fp8, kind="ExternalInput")
    wk8_in = nc.dram_tensor("wk8_in", [128, 2, KD], fp8, kind="ExternalInput")
    wk_t = nc.dram_tensor("wk_t", [HD, KD], f32, kind="ExternalInput")
    # pretransposed on host: cols 0-3 64*Qb, 4-7 64*Kb, 8-11 h_mat
    bias_cols = nc.dram_tensor("bias_cols", [128, 12], f32, kind="ExternalInput")
    # pretransposed: cols 0-15 mask_p {0,1}; 16-31 valid {0,1}; 32.. mask_v/L packed
    mask_cols = nc.dram_tensor("mask_cols", [128, 32 + NQT], f32, kind="ExternalInput")
    ident_in = nc.dram_tensor("ident_in", [128, 128], f32, kind="ExternalInput")
    kb_div = nc.dram_tensor("kb_div", [KD], f32, kind="ExternalInput")  # Kb/128
    out = nc.dram_tensor("out", [1, KD], f32, kind="ExternalOutput")

    NHC = HD // 128   # 2 h chunks
    NKC = KD // 128   # 4 k chunks
    NLT = LP // 128   # 16 l tiles
    NQC = LP // 512   # 4 q chunks

    with tile.TileContext(nc) as tc:
        import contextlib
        ctx = contextlib.ExitStack()
        with ctx:
            singles = ctx.enter_context(tc.tile_pool(name="singles", bufs=1))
            stage = ctx.enter_context(tc.tile_pool(name="stage", bufs=4))
            wsmall = ctx.enter_context(tc.tile_pool(name="wsmall", bufs=16))
            epool = ctx.enter_context(tc.tile_pool(name="epool", bufs=3))
            pfc = ctx.enter_context(tc.tile_pool(name="pfc", bufs=2, space="PSUM"))
            pg = ctx.enter_context(tc.tile_pool(name="pg", bufs=2, space="PSUM"))
            ps = ctx.enter_context(tc.tile_pool(name="ps", bufs=2, space="PSUM"))

            # ---- constants / params ----
            ident = singles.tile([128, 128], f32)
            nc.sync.dma_start(ident, ident_in[:])
            warm_ps = pfc.tile([128, 512], f32, tag="fc")
            nc.tensor.transpose(warm_ps[:, 0:128], ident, ident)

            wq8 = singles.tile([128, 2, KD], fp8)
            nc.sync.dma_start(wq8, wq8_in[:])
            wk8 = singles.tile([128, 2, KD], fp8)
            nc.sync.dma_start(wk8, wk8_in[:])
            wk_st = singles.tile([128, NHC, KD], f32, tag="wk_st")
            nc.sync.dma_start(wk_st, wk_t[:].rearrange("(c p) k -> p c k", p=128))
            wk_sb = singles.tile([128, NHC, KD], f32r)  # for fp32r FC-nat
            nc.vector.tensor_copy(wk_sb, wk_st)

            bcols = singles.tile([128, 12], f32)
            nc.sync.dma_start(bcols, bias_cols[:])
            # prime ACT's clock on the bias DMA so FC evacuations only wait on PE
            bprime = singles.tile([128, 12], f32)
            nc.scalar.copy(bprime, bcols)
            qb64_col = bcols[:, 0:NKC]              # 64*Qb
            kb64_col = bcols[:, NKC : 2 * NKC]       # 64*Kb
            h_col = bcols[:, 2 * NKC : 3 * NKC]      # h_mat

            mcols = singles.tile([128, 32 + NQT], f32)
            nc.sync.dma_start(mcols, mask_cols[:])
            mp_col = mcols[:, 0:NLT]          # numerator mask, {0,1}
            valid_col = mcols[:, NLT : 2 * NLT]
            mv_col = mcols[:, 2 * NLT :]      # output mask, {0,1/L}, packed

            # reduction stationary, DoubleRow-paired over u-tile pairs:
            # rbuf8[p, ko, ltp, m]: ko = which u-tile of the pair, m = [valid, mask_p]
            rbuf8 = singles.tile([128, 2, NLT // 2, 2], fp8)
            for ko in range(2):
                nc.vector.tensor_copy(rbuf8[:, ko, :, 0], valid_col[:, ko::2])
                nc.vector.tensor_copy(rbuf8[:, ko, :, 1], mp_col[:, ko::2])

            # Kb/128 broadcast to all partitions (for the FC-nat bias matmul)
            kbd_st = singles.tile([128, KD], f32)
            nc.gpsimd.dma_start(kbd_st, kb_div[:].partition_broadcast(128))
            kbd_bc = singles.tile([128, KD], f32r)
            nc.vector.tensor_copy(kbd_bc, kbd_st)
            ones_st = singles.tile([128, 128], f32)
            nc.vector.memset(ones_st, 1.0)
            ones_t = singles.tile([128, 128], f32r)
            nc.vector.tensor_copy(ones_t, ones_st)

            # ---- input transposes via XBAR (uint16 = fp8 h-pairs), chunked
            # by 512-row pieces and interleaved with FC-T ----
            p8t = singles.tile([128, LP], u16)
            r8t = singles.tile([128, NQP], u16)
            # fp8 views with the h-pair as the DoubleRow interleave dim
            p8v = p8t[:].bitcast(fp8).rearrange("p (l two) -> p two l", two=2)
            r8v = r8t[:].bitcast(fp8).rearrange("p (l two) -> p two l", two=2)
            rt_sb = singles.tile([128, NHC, NQP], f32r)
            ut_bf = singles.tile([128, NKC, LP], bf16)
            ut8 = singles.tile([128, NKC, LP], fp8)
            vt8 = singles.tile([128, NKC, NQP], fp8)
            for vc in range(NQC):
                sl = slice(vc * 512, (vc + 1) * 512)
                nc.sync.dma_start_transpose(p8t[:, sl], p8_in[sl, :])
                if vc < NQCh:
                    nc.sync.dma_start_transpose(r8t[:, sl], r8_in[sl, :])
                # FC-T for this l-chunk (fp8 DoubleRow, K=256 in one matmul)
                for kc in range(NKC):
                    pm = pfc.tile([128, 512], f32, tag="fc")
                    nc.tensor.matmul(
                        pm,
                        lhsT=wq8[:, :, kc * 128 : (kc + 1) * 128],
                        rhs=p8v[:, :, sl],
                        perf_mode=DR,
                    )
                    nc.vector.tensor_scalar(
                        ut_bf[:, kc, sl], pm, qb64_col[:, kc : kc + 1], 0.0,
                        mybir.AluOpType.add, mybir.AluOpType.max,
                    )
                    nc.gpsimd.tensor_scalar_mul(
                        ut8[:, kc, sl], ut_bf[:, kc, sl], h_col[:, kc : kc + 1]
                    )
                    if vc < NQCh:
                        pm2 = pfc.tile([128, 512], f32, tag="fc")
                        nc.tensor.matmul(
                            pm2,
                            lhsT=wk8[:, :, kc * 128 : (kc + 1) * 128],
                            rhs=r8v[:, :, sl],
                            perf_mode=DR,
                        )
                        nc.scalar.activation(
                            vt8[:, kc, sl], pm2, AF.Relu, bias=kb64_col[:, kc : kc + 1]
                        )

            # ---- FC-nat (fp32r value chain), emitted interleaved with the G
            # loop so the PE fills the exp-bound pipeline bubbles ----
            vnat = singles.tile([128, NQT, KD], f32r)

            r_nat3 = r_f32[:].rearrange("(t p) h -> t p h", p=128)

            def r_transpose(lt):
                nat = stage.tile([128, HD], f32, tag="nat")
                nc.sync.dma_start(nat, r_nat3[lt])
                for hc in range(NHC):
                    tp = pfc.tile([128, 512], f32, tag="fc")
                    nc.tensor.transpose(
                        tp[:, 0:128], nat[:, hc * 128 : (hc + 1) * 128], ident
                    )
                    nc.vector.tensor_copy(
                        rt_sb[:, hc, lt * 128 : (lt + 1) * 128], tp[:, 0:128]
                    )

            def fc_nat(qt):
                pm = pfc.tile([128, 512], f32, tag="fc")
                for hc in range(NHC):
                    nc.tensor.matmul(
                        pm,
                        lhsT=rt_sb[:, hc, qt * 128 : (qt + 1) * 128],
                        rhs=wk_sb[:, hc, :],
                        start=(hc == 0),
                        stop=False,
                    )
                nc.tensor.matmul(
                    pm, lhsT=ones_t, rhs=kbd_bc[:],
                    start=False, stop=True, skip_group_check=True,
                )
                nc.vector.tensor_scalar_max(vnat[:, qt, :], pm, 0.0)

            # ---- w = mask_v/L * S_w/S_all, as column tiles ----
            wcol = singles.tile([128, NQT], f32r)
            s_sb = singles.tile([2, NQCh, 512], f32)

            def w_math(qc):
                for j in range(4):
                    qt = qc * 4 + j
                    st_ps = pfc.tile([128, 512], f32, tag="fc")
                    nc.tensor.transpose(
                        st_ps[:, 0:2], s_sb[:, qc, j * 128 : (j + 1) * 128],
                        ident[:2, :2],
                    )
                    s2 = wsmall.tile([128, 2], f32, tag="s2")
                    nc.scalar.copy(s2, st_ps[:, 0:2])
                    rcp = wsmall.tile([128, 1], f32, tag="rcp")
                    nc.vector.reciprocal(rcp, s2[:, 0:1])
                    nc.vector.tensor_mul(rcp, rcp, s2[:, 1:2])
                    nc.vector.tensor_mul(
                        wcol[:, qt : qt + 1], rcp, mv_col[:, qt : qt + 1]
                    )

            # ---- G (fp8 DoubleRow) + exp + fp8 DoubleRow reduction ----
            spans = []   # (first chunk idx, chunks in span)
            c0 = 0
            while c0 < NQCh:
                wc = min(2, NQCh - c0)
                spans.append((c0, wc))
                c0 += wc
            for si, (c0, wc) in enumerate(spans):
                s_list = [
                    ps.tile([2, 512], f32, tag="s", name=f"s_ps_{si}_{h}")
                    for h in range(wc)
                ]
                wq = wc * 512
                for ltp in range(NLT // 2):    # pairs of u tiles
                    et = epool.tile([128, 2, 1024], fp8, tag="e")
                    for sub in range(2):
                        lt = 2 * ltp + sub
                        gp = pg.tile([128, 1024], f32, tag="g")
                        for half in range(wc):
                            qs = slice((c0 + half) * 512, (c0 + half + 1) * 512)
                            for j in range(2):
                                nc.tensor.matmul(
                                    gp[:, half * 512 : (half + 1) * 512],
                                    lhsT=ut8[:, 2 * j : 2 * j + 2, lt * 128 : (lt + 1) * 128],
                                    rhs=vt8[:, 2 * j : 2 * j + 2, qs],
                                    start=(j == 0),
                                    stop=(j == 1),
                                    perf_mode=DR,
                                )
                        nc.scalar.activation(
                            et[:, sub, :wq], gp[:, :wq], AF.Exp,
                            scale=1.0 / (WSCALE * WSCALE),
                        )
                    for half in range(wc):
                        nc.tensor.matmul(
                            s_list[half],
                            lhsT=rbuf8[:, :, ltp, :],
                            rhs=et[:, :, half * 512 : (half + 1) * 512],
                            start=(ltp == 0), stop=(ltp == NLT // 2 - 1),
                            perf_mode=DR,
                            skip_group_check=True,
                        )
                    s_idx = si * (NLT // 2) + ltp
                    if s_idx < NQT:
                        r_transpose(s_idx)
                    if 1 <= s_idx <= NQT - 1:
                        fc_nat(s_idx - 1)
                for half in range(wc):
                    nc.scalar.copy(s_sb[:, c0 + half, :], s_list[half])
                    w_math(c0 + half)


            fc_nat(NQT - 1)

            # ---- contrib = w^T @ Vnat ----
            c_ps = pfc.tile([128, 512], f32, tag="fc")
            for qt in range(NQT):
                nc.tensor.matmul(
                    c_ps[0:1, :],
                    lhsT=wcol[:, qt : qt + 1],
                    rhs=vnat[:, qt, :],
                    start=(qt == 0),
                    stop=(qt == NLT - 1),
                )
            out_sb = singles.tile([1, KD], f32)
            nc.scalar.copy(out_sb, c_ps[0:1, :])
            nc.gpsimd.dma_start(out[:], out_sb)

    nc.finalize()
    return nc


def _get_nc(nqch=4):
    if nqch not in _NC_CACHE:
        _NC_CACHE[nqch] = _build_nc(nqch)
    return _NC_CACHE[nqch]


def kernel(**inputs) -> np.ndarray:
    import ml_dtypes
    from concourse.bass_utils import run_bass_kernel_spmd

    X = np.asarray(inputs["X"], dtype=np.float32)
    Y = np.asarray(inputs["Y"], dtype=np.float32)
    m1 = np.asarray(inputs["mask1"], dtype=np.float32)
    m2 = np.asarray(inputs["mask2"], dtype=np.float32)
    Qv = np.asarray(inputs["Qv"], dtype=np.float32)
    Qg = np.float32(np.asarray(inputs["Qg"]))
    Qb = np.asarray(inputs["Qb"], dtype=np.float32)
    Kv = np.asarray(inputs["Kv"], dtype=np.float32)
    Kg = np.float32(np.asarray(inputs["Kg"]))
    Kb = np.asarray(inputs["Kb"], dtype=np.float32)
    hm = np.asarray(inputs["h_mat"], dtype=np.float32)
    gamma = np.asarray(inputs["gamma"], dtype=np.float32)
    beta = np.asarray(inputs["beta"], dtype=np.float32)

    import ml_dtypes as _mld

    Wq = (Qg / np.float32(np.linalg.norm(Qv))) * Qv  # [KD, HD]
    Wk = (Kg / np.float32(np.linalg.norm(Kv))) * Kv
    wk_t = np.ascontiguousarray(Wk.T)
    # 64*W^T reshaped so rows pair consecutive h for the DoubleRow interleave
    wq8_in = np.ascontiguousarray(
        (WSCALE * Wq.T).reshape(128, 2, KD).astype(_mld.float8_e4m3)
    )
    wk8_in = np.ascontiguousarray(
        (WSCALE * Wk.T).reshape(128, 2, KD).astype(_mld.float8_e4m3)
    )

    bias_cols = np.ascontiguousarray(
        np.concatenate(
            [(WSCALE * Qb).reshape(4, 128), (WSCALE * Kb).reshape(4, 128),
             hm.reshape(4, 128)], axis=0
        ).T
    ).astype(np.float32)  # [128, 12]
    kb_div = (Kb / 128.0).astype(np.float32)
    ident = np.eye(128, dtype=np.float32)

    def padded(v2000, scale=1.0):
        p = np.zeros((LP,), np.float32)
        p[:L] = v2000 * scale
        return p.reshape(16, 128)

    valid = padded(np.ones(L, np.float32))

    def pad_seq(s):
        p = np.zeros((LP, HD), np.float32)
        p[:L] = s
        return p

    # Only q columns with mask_v > 0 contribute; permute them to the front and
    # size the computed q window (in 1024-col pairs) to cover every valid
    # column across all 8 cores.
    units = []
    max_nv = 0
    for b in range(B):
        for m in range(2):
            if m == 0:
                P, R, mp, mv = X[b], Y[b], m1[b], m2[b]
            else:
                P, R, mp, mv = Y[b], X[b], m2[b], m1[b]
            perm = np.argsort(mv <= 0, kind="stable")
            max_nv = max(max_nv, int((mv > 0).sum()))
            units.append((P, R, mp, mv, perm))
    nqch = min(4, max(2, -(-max_nv // 512)))
    NQP = 512 * nqch
    NQT = NQP // 128

    in_maps = []
    for P, R, mp, mv, perm in units:
        nperm = min(NQP, L)
        Rp = np.zeros((NQP, HD), np.float32)
        Rp[:nperm] = R[perm[:nperm]]
        mvp = np.zeros((NQP,), np.float32)
        mvp[:nperm] = mv[perm[:nperm]] * (1.0 / L)
        mask_cols = np.ascontiguousarray(
            np.concatenate(
                [padded(mp), valid, mvp.reshape(NQT, 128)], axis=0
            ).T
        ).astype(np.float32)  # [128, 32 + NQT]
        p8 = pad_seq(P).astype(ml_dtypes.float8_e4m3).view(np.uint16)
        r8 = Rp.astype(ml_dtypes.float8_e4m3).view(np.uint16)
        in_maps.append(
            {
                "p8_in": p8,
                "r8_in": r8,
                "r_f32": Rp,
                "wq8_in": wq8_in,
                "wk8_in": wk8_in,
                "wk_t": wk_t,
                "bias_cols": bias_cols,
                "mask_cols": mask_cols,
                "ident_in": ident,
                "kb_div": kb_div,
            }
        )

    nc = _get_nc(nqch)
    res = run_bass_kernel_spmd(nc, in_maps, core_ids=list(range(NCORES)))
    contribs = np.stack([r["out"][0] for r in res.results]).astype(np.float64)

    pooled = contribs[0::2] + contribs[1::2]  # [B, KD]
    mu = pooled.mean(axis=0)
    var = pooled.var(axis=0)
    outv = gamma * (pooled - mu) / np.sqrt(var + EPS) + beta
    return outv.astype(np.float32)

